# revision 26
# baseline (speedup 1.0000x reference)
"""Trainium2 Bass kernel for AttentionalFactorizationMachine.

kernel(**inputs) takes FULL unsharded inputs, returns FULL [2048, 1] output.
Internally: data-parallel over 8 NeuronCores (batch sharded, weights
replicated), one SPMD Bass program.

Per-core algorithm (256 items, 780 pairs padded to 784):
  out[b] = (sum_p E_p * g_p) / (sum_p E_p) + fc_b
    E_p = exp(l_p)                       [proj_b dropped: softmax-invariant]
    l_p = proj_w . relu(attn_w^T (x_i*x_j) + attn_b)
    g_p = fc_w . (x_i*x_j)
  Device computes num/den per item; host does the final divide + fc_b.

Layouts (SBUF [partition, free]):
  X_T [ (half,d)=128, (field,b_q)=40*128 ] fp16  HOST-packed (one DMA)
  ip  [ (half,d)=128, (pair_loc,b_q)     ] fp16  DVE broadcast tensor_mul (2x)
Pipeline per supertile (16 pairs):
  L1:  8 concurrent matmuls (2 b-halves x 4 col-groups, tile_position),
       lhsT=attn_w -> ps_t2 [(cgrp,a)=128, 1024]
  ACT: fused bias+relu psum->SBUF hs2 fp16
  L2': hs2 128-col chunks as lhsT (transposed trick), rhs=proj4 [128,4]
       -> lg_ps [b_q=128, pair-cols]   (dense logits)  -- LDW ~free (FWL)
  g:   ip 128-col chunks as lhsT, rhs=fcw2 [128,2]
       -> g_ps [b_q=128, pair-cols]    (dense values)
Software pipeline: the L2'/g matmuls of supertile s are EMITTED after the
L1 of supertile s+1.  They read hs2 (the ACT relu output); without the
delay the in-order PE queue stalls ~1us on the ACT every supertile --
that stall, not matmul cost, dominated earlier versions.
Per round (16 supertiles): ACT exp -> E + accum_out (den partials);
DVE scalar_tensor_tensor(E*g) -> num partials (chained).
"""

import numpy as np

B, F, D, A = 2048, 40, 64, 32
N_CORES = 8
BC = B // N_CORES          # 256 items per core
BQ = 128                   # items per half
N_HALF = 2
PAIRS = F * (F - 1) // 2   # 780
ST_PAIRS = 16
N_ST = (PAIRS + ST_PAIRS - 1) // ST_PAIRS       # 49
PAIRS_PAD = N_ST * ST_PAIRS                     # 784
ROUND_ST = 16
N_ROUNDS = (N_ST + ROUND_ST - 1) // ROUND_ST    # 4
NEG_BIG = -1.0e30
IP_BUFS = 4
H_BUFS = 4
WP_COLS = 166      # [aw 32 | proj4 4 | wB 64+2(shared fc) | thr 64]
A_ST = 36          # supertiles on the A pipeline; pairs >= 16*A_ST go via B
B_CH = 14          # pairs per B chunk (7 per PSUM bank, 66 fp32 cols each)

# pairs ordered by (j, i): long i-runs per j => few DVE ops
_ROWJ = np.array([i for j in range(1, F) for i in range(j)], np.int32)
_COLJ = np.array([j for j in range(1, F) for i in range(j)], np.int32)


def _pos(p_local: int) -> int:
    # L2' chunk j with rhs col m' produces pair_local=4m'+j at col 4j+m'
    return 4 * (p_local % 4) + p_local // 4


def _b_chunks(a_st):
    """B-phase chunks [(start_pair, npairs), ...] covering [16*a_st, 780)."""
    out, p = [], ST_PAIRS * a_st
    while p < PAIRS:
        n = min(B_CH, PAIRS - p)
        out.append((p, n))
        p += n
    return out


def _patch_tile_drain():
    """This walrus build accepts only ONE sync wait per instruction; split the
    TileContext exit drain into a chain of single-wait drains."""
    import bass_rust
    import concourse.tile as tile_mod
    from concourse.tile import TileContext

    if getattr(TileContext, "_drain_patched", False):
        return

    def _drain_and_barrier(self, tick_clock, wait_clock):
        drain_inst = self.nc.sync.drain()
        wait_clock.add_sem_waits(
            drain_inst.ins, tile_mod.ScopedClock({None: tick_clock.global_clock})
        )
        si = drain_inst.ins.sync_info
        if si is not None and len(si.on_wait) > 1:
            waits = list(si.on_wait)
            drain_inst.ins.sync_info = bass_rust.SyncInfo(
                on_wait=[waits[0]], on_update=list(si.on_update)
            )
            for w in waits[1:]:
                extra = self.nc.sync.drain()
                extra.ins.sync_info = bass_rust.SyncInfo(on_wait=[w], on_update=[])

    TileContext._drain_and_barrier = _drain_and_barrier
    TileContext._drain_patched = True


def _split_multiwait(nc):
    """Walrus here allows ONE sync wait per instruction: move surplus waits
    onto same-engine NoOps inserted immediately before the instruction."""
    import concourse.mybir as mybir

    for f in nc.m.functions:
        for blk in f.blocks:
            il = blk.instructions
            idx = 0
            while idx < len(il):
                inst = il[idx]
                si = inst.sync_info
                if si is not None and len(si.on_wait) > 1:
                    waits = list(si.on_wait)
                    inst.sync_info = mybir.SyncInfo(
                        on_wait=[waits[-1]], on_update=list(si.on_update)
                    )
                    for k, w in enumerate(waits[:-1]):
                        nop = mybir.InstNoOp(
                            name=f"{inst.name}_w{k}",
                            sync_info=mybir.SyncInfo(on_wait=[w], on_update=[]),
                            bass_nofuse=True,
                            engine=inst.engine,
                        )
                        il.insert(idx, nop)
                        idx += 1
                idx += 1


def _seg_of_st():
    """Per-supertile ip-build segments: ("tt", j, i0, i1, p_local0) or
    ("pad", 0, 0, n, pl); i-runs split only at supertile boundaries."""
    seg_of_st = []
    for s in range(N_ST):
        p_lo, p_hi = s * ST_PAIRS, (s + 1) * ST_PAIRS
        segs, p = [], p_lo
        while p < p_hi:
            if p < PAIRS:
                i, j = int(_ROWJ[p]), int(_COLJ[p])
                run = min(p_hi, PAIRS, p + (j - i)) - p  # i..j-1 within j-row
                segs.append(("tt", j, i, i + run, p - p_lo))
                p += run
            else:
                segs.append(("pad", 0, 0, p_hi - p, p - p_lo))
                p = p_hi
        seg_of_st.append(segs)
    return seg_of_st


def build_core_program(split_waits=True, repeat=1, skip=(), a_st=A_ST, na=16):
    """The single-core SPMD Bass program (identical on all 8 cores).

    a_st: supertiles handled by the A pipeline (relu on ACT, L2'/g LDW
    matmuls); pairs >= 16*a_st go through the B pipeline (per-pair
    LDW+66col matmul, sign-folded clamp on DVE).  na: number of
    non-negative proj_w entries (clamp split point; affects values only,
    not timing).
    """
    import concourse.bass as bass
    import concourse.mybir as mybir
    from concourse.tile import TileContext

    _patch_tile_drain()
    dt = mybir.dt
    AF = mybir.ActivationFunctionType
    ALU = mybir.AluOpType

    nc = bass.Bass()
    # x arrives host-transposed + fp16: [(half,d)=128, (field, b_q)]
    x_in = nc.dram_tensor("x", [128, F * BQ], dt.float16,
                          kind="ExternalInput")
    # wp16 [128, 134] fp16 = [aw(32) | proj4(4) | wB(64) | fcw2(2) | thr(32)]
    # wp32 [128, 1] fp32 = attn_b tiled 4x
    wp16_in = nc.dram_tensor("wp16", [128, WP_COLS], dt.float16,
                             kind="ExternalInput")
    wp32_in = nc.dram_tensor("wp32", [128, 1], dt.float32, kind="ExternalInput")
    out_t = nc.dram_tensor("out", [BC, 2], dt.float32, kind="ExternalOutput")

    seg_of_st = _seg_of_st()
    n_rounds_a = (a_st + ROUND_ST - 1) // ROUND_ST
    bchunks = _b_chunks(a_st)
    nb_ch = len(bchunks)
    # ragged 2-bank chunk needs a second num partial column
    extra_col = 1 if any(7 < n < B_CH for _, n in bchunks) else 0
    npart = n_rounds_a + nb_ch + extra_col

    with TileContext(nc) as tc:
        with (
            tc.tile_pool(name="const", bufs=1) as cpool,
            tc.tile_pool(name="xstage", bufs=1) as xpool,
            tc.tile_pool(name="ip", bufs=IP_BUFS) as ippool,
            tc.tile_pool(name="hbuf", bufs=H_BUFS) as hpool,
            tc.tile_pool(name="escr", bufs=2) as epool,
            tc.tile_pool(name="bbuf", bufs=3) as bpool,
            tc.tile_pool(name="acc", bufs=1) as apool,
            tc.tile_pool(name="pst", bufs=2, space="PSUM") as pst,
            tc.tile_pool(name="plg", bufs=2, space="PSUM") as plg,
            tc.tile_pool(name="pgv", bufs=2, space="PSUM") as pgv,
        ):
            import contextlib
            loop_cm = (tc.For_i(0, repeat, 1) if repeat > 1
                       else contextlib.nullcontext())
            with loop_cm:
                # ---------- weights + x: 3 DMAs ----------
                wp16 = cpool.tile([128, WP_COLS], dt.float16)
                nc.sync.dma_start(wp16[:], wp16_in[:])
                ab = cpool.tile([128, 1], dt.float32)
                nc.sync.dma_start(ab[:], wp32_in[:])
                aw = wp16[:, 0:A]
                proj4 = wp16[:, A:A + 4]
                wB = wp16[:, 36:102]        # [wtil(64, (k,hh)) | fc(2)]
                fcw2 = wp16[:, 100:102]
                thr_t = wp16[:, 102:166]    # clamp thresholds, all-partition
                xt = xpool.tile([128, F * BQ], dt.float16, tag="xt")
                nc.sync.dma_start(xt[:], x_in[:])

                # ---------- accumulators ----------
                # partials laid out [128, (h, r)]: col h*npart + r
                num_parts = apool.tile([128, 2 * npart], dt.float32)
                den_parts = apool.tile([128, 2 * npart], dt.float32)
                nc.vector.memset(num_parts[:], 0.0)
                nc.vector.memset(den_parts[:], 0.0)

                # round r -> (lg_ps, g_ps); delayed by the sw pipeline
                round_ps = {}
                pend = None  # (s, r, sl, ip, hs2)

                def emit_tail(s0, r0, sl0, ip0, hs20):
                    lg_ps, g_ps = round_ps[r0]
                    # ---- L2' dense logits (lhsT = hs2 data, LDW ~free) ----
                    for jj in (range(8)
                               if not ({"lg", "l2"} & set(skip)) else []):
                        h, j = jj // 4, jj % 4
                        c0 = 32 * sl0 + 16 * h + 4 * j
                        nc.tensor.matmul(
                            lg_ps[:, c0:c0 + 4],
                            hs20[:, 512 * h + 128 * j:512 * h + 128 * (j + 1)],
                            proj4,
                            start=True, stop=True,
                        )
                    # ---- g dense ----
                    for pl in (range(ST_PAIRS)
                               if not ({"lg", "g"} & set(skip)) else []):
                        c0 = 32 * sl0 + 2 * _pos(pl)
                        nc.tensor.matmul(
                            g_ps[:, c0:c0 + 2],
                            ip0[:, BQ * pl:BQ * (pl + 1)],
                            fcw2,
                            start=True, stop=True,
                        )
                    st1 = min((r0 + 1) * ROUND_ST, a_st)
                    if s0 != st1 - 1 or "red" in skip:
                        return
                    n_st_r = st1 - r0 * ROUND_ST
                    # ---- poison pad logits -> exp gives 0 ----
                    if st1 == N_ST and PAIRS_PAD > PAIRS:
                        sl = (N_ST - 1) - r0 * ROUND_ST
                        for h in range(N_HALF):
                            base = 32 * sl + 16 * h
                            nc.vector.memset(
                                lg_ps[:, base + 3:base + 16:4], NEG_BIG
                            )
                    # ---- exp (den) + E*g reduce (num) ----
                    e_sb = epool.tile([128, 512], dt.float32, tag="E",
                                      name=f"e_{r0}")
                    tt_scr = epool.tile([128, 256], dt.float32, tag="ttscr",
                                        name=f"tt_{r0}")
                    for h in range(N_HALF):
                        l_ap = lg_ps[:].rearrange(
                            "q (s hh c) -> q c hh s", hh=2, c=16
                        )[:, :, h, 0:n_st_r]
                        e_ap = e_sb[:].rearrange(
                            "q (s hh c) -> q c hh s", hh=2, c=16
                        )[:, :, h, 0:n_st_r]
                        g_ap = g_ps[:].rearrange(
                            "q (s c hh) -> q c hh s", c=16, hh=2
                        )[:, :, h, 0:n_st_r]
                        col = h * npart + r0
                        nc.scalar.activation(
                            e_ap, l_ap, AF.Exp,
                            accum_out=den_parts[:, col:col + 1],
                        )
                        nc.vector.scalar_tensor_tensor(
                            out=tt_scr[:, 0:16 * n_st_r].rearrange(
                                "q (c s) -> q c s", s=n_st_r
                            ),
                            in0=e_ap,
                            scalar=1.0,
                            in1=g_ap,
                            op0=ALU.mult,
                            op1=ALU.mult,
                            accum_out=num_parts[:, col:col + 1],
                        )

                def emit_b_tail(bp, p0, n, ci):
                    """Clamp+reduce+exp+num for one B chunk of n pairs."""
                    hb = hpool.tile([128, 1024], dt.float16, tag="h",
                                    name=f"hb_{ci}")
                    banks = [(0, min(n, 7)), (1, max(n - 7, 0))]
                    for b, nbk in banks:
                        if nbk:
                            nc.scalar.activation(
                                hb[:, 512 * b:512 * b + 66 * nbk],
                                bp[:, 512 * b:512 * b + 66 * nbk],
                                AF.Copy)
                    # sign-folded clamp: col k<na -> max(h~, thr), else min
                    # wB/hb/hc h-cols are (k, hh)-interleaved: col 2k+hh
                    hc = bpool.tile([128, 64 * B_CH], dt.float16, tag="hc",
                                    name=f"hc_{ci}")
                    for b, nbk in banks:
                        if not nbk:
                            continue
                        h_in = hb[:, 512 * b:512 * b + 66 * nbk].rearrange(
                            "q (pp c) -> q pp c", c=66)
                        h_out = hc[:, 448 * b:448 * b + 64 * nbk].rearrange(
                            "q (pp c) -> q pp c", c=64)
                        for k0, k1, op in ((0, na, ALU.max),
                                           (na, 32, ALU.min)):
                            if k1 <= k0:
                                continue
                            nc.vector.scalar_tensor_tensor(
                                out=h_out[:, :, 2 * k0:2 * k1],
                                in0=h_in[:, :, 2 * k0:2 * k1],
                                scalar=1.0,
                                in1=thr_t[:, 2 * k0:2 * k1].rearrange(
                                    "q (o c) -> q o c", o=1
                                ).broadcast_to((128, nbk, 2 * (k1 - k0))),
                                op0=ALU.mult,
                                op1=op,
                            )
                    # l[q, (slot, hh)] = sum over the 32 clamped cols
                    rd = bpool.tile([128, 2 * B_CH], dt.float32, tag="rd",
                                    name=f"rd_{ci}")
                    for h in range(N_HALF):
                        nc.vector.tensor_reduce(
                            rd[:, 0:2 * n].rearrange(
                                "q (sl hh) -> q sl hh", hh=2)[:, :, h],
                            hc[:, 0:64 * n].rearrange(
                                "q (sl kh) -> q sl kh", kh=64)[:, :, h::2],
                            mybir.AxisListType.X, ALU.add,
                        )
                    if "red" in skip:
                        return
                    e_b = bpool.tile([128, 2 * B_CH], dt.float32, tag="eb",
                                     name=f"eb_{ci}")
                    tt_b = bpool.tile([128, B_CH], dt.float32, tag="ttb",
                                      name=f"ttb_{ci}")
                    for h in range(N_HALF):
                        col = h * npart + n_rounds_a + ci
                        e_ap = e_b[:, 0:2 * n].rearrange(
                            "q (sl hh) -> q sl hh", hh=2)[:, :, h]
                        nc.scalar.activation(
                            e_ap,
                            rd[:, 0:2 * n].rearrange(
                                "q (sl hh) -> q sl hh", hh=2)[:, :, h],
                            AF.Exp,
                            accum_out=den_parts[:, col:col + 1],
                        )
                        if n == B_CH:
                            nc.vector.scalar_tensor_tensor(
                                out=tt_b[:, 0:B_CH].rearrange(
                                    "q (b pp) -> q b pp", b=2),
                                in0=e_b[:, 0:2 * n].rearrange(
                                    "q (b pp hh) -> q b pp hh", b=2, hh=2
                                )[:, :, :, h],
                                scalar=1.0,
                                in1=hb[:].rearrange(
                                    "q (b r) -> q b r", b=2
                                )[:, :, 64 + h:64 + h + 66 * 6 + 1:66],
                                op0=ALU.mult, op1=ALU.mult,
                                accum_out=num_parts[:, col:col + 1],
                            )
                        else:
                            for b, nbk in banks:
                                if not nbk:
                                    continue
                                colb = col if b == 0 else h * npart + npart - 1
                                nc.vector.scalar_tensor_tensor(
                                    out=tt_b[:, 0:nbk],
                                    in0=e_b[:, 0:2 * n].rearrange(
                                        "q (sl hh) -> q sl hh", hh=2
                                    )[:, 7 * b:7 * b + nbk, h],
                                    scalar=1.0,
                                    in1=hb[:, 512 * b + 64 + h:
                                           512 * b + 64 + h
                                           + 66 * (nbk - 1) + 1:66],
                                    op0=ALU.mult, op1=ALU.mult,
                                    accum_out=num_parts[:, colb:colb + 1],
                                )

                b_done = []   # chunks awaiting their tail (delay-by-one)
                b_cur = None  # [bp_tile, start_pair, chunk_idx]

                for s in range(N_ST):
                    is_a = s < a_st
                    r, sl = divmod(s, ROUND_ST)
                    if is_a and sl == 0:
                        lg_ps = plg.tile([128, 512], dt.float32, tag="lg",
                                         name=f"lg_{r}")
                        g_ps = pgv.tile([128, 512], dt.float32, tag="gv",
                                        name=f"g_{r}")
                        round_ps[r] = (lg_ps, g_ps)

                    # ---- ip build ----
                    ip = ippool.tile([128, ST_PAIRS * BQ], dt.float16,
                                     tag="ip", name=f"ip_{s}")
                    segs = seg_of_st[s] if "ip" not in skip else []
                    if not is_a:
                        segs = [sg for sg in segs if sg[0] == "tt"]
                    if "ip" in skip:
                        nc.vector.tensor_mul(
                            ip[:, 0:BQ], xt[:, 0:BQ], xt[:, BQ:2 * BQ])
                    for kind, j, i0, i1, pl in segs:
                        if kind == "pad":
                            nc.vector.memset(
                                ip[:, pl * BQ:(pl + i1 - i0) * BQ], 0.0)
                            continue
                        nii = i1 - i0
                        nc.vector.tensor_mul(
                            ip[:, pl * BQ:(pl + nii) * BQ].rearrange(
                                "p (j q) -> p j q", j=nii
                            ),
                            xt[:, i0 * BQ:i1 * BQ].rearrange(
                                "p (j q) -> p j q", j=nii
                            ),
                            xt[:, j * BQ:(j + 1) * BQ].rearrange(
                                "p (o q) -> p o q", o=1
                            ).broadcast_to((128, nii, BQ)),
                        )

                    if is_a:
                        # ---- L1: 8 concurrent matmuls, one 2-bank tile ----
                        ps_t2 = pst.tile([128, 1024], dt.float32, tag="t",
                                         name=f"ps_t_{s}")
                        for h in (range(N_HALF) if "l1" not in skip else []):
                            for c in range(4):
                                nc.tensor.matmul(
                                    ps_t2[32 * c:32 * (c + 1),
                                          512 * h:512 * (h + 1)],
                                    aw[64 * h:64 * (h + 1), :],
                                    ip[64 * h:64 * (h + 1),
                                       512 * c:512 * (c + 1)],
                                    start=True, stop=True,
                                    tile_position=(64 * h, 32 * c),
                                )

                        # ---- relu + bias ----
                        hs2 = hpool.tile([128, 1024], dt.float16, tag="h",
                                         name=f"hs_{s}")
                        if "relu" in skip:
                            nc.scalar.activation(
                                hs2[:, 0:16], ps_t2[:, 0:16], AF.Relu,
                                bias=ab[:], scale=1.0)
                        else:
                            nc.scalar.activation(
                                hs2[:], ps_t2[:], AF.Relu, bias=ab[:],
                                scale=1.0)

                        # ---- delayed L2'/g of the PREVIOUS supertile ----
                        if pend is not None:
                            emit_tail(*pend)
                        pend = (s, r, sl, ip, hs2)
                    elif "l1" not in skip:
                        # ---- B phase: per-pair LDW+66col matmul ----
                        if pend is not None:
                            emit_tail(*pend)
                            pend = None
                        for pl in range(ST_PAIRS):
                            pair = ST_PAIRS * s + pl
                            if pair >= PAIRS:
                                break
                            bi = pair - ST_PAIRS * a_st
                            ci, slot = divmod(bi, B_CH)
                            if slot == 0:
                                bp = pst.tile([128, 1024], dt.float32,
                                              tag="t", name=f"bp_{ci}")
                                b_cur = [bp, pair, ci]
                            c0 = 512 * (slot // 7) + 66 * (slot % 7)
                            nc.tensor.matmul(
                                b_cur[0][:, c0:c0 + 66],
                                ip[:, BQ * pl:BQ * (pl + 1)],
                                wB,
                                start=True, stop=True,
                            )
                            if slot == B_CH - 1 or pair == PAIRS - 1:
                                # tail of the PREVIOUS chunk (delay-by-one)
                                while b_done:
                                    emit_b_tail(*b_done.pop(0))
                                b_done.append(
                                    (b_cur[0], b_cur[1],
                                     pair - b_cur[1] + 1, b_cur[2]))

                # flush the last supertile / B chunks
                if pend is not None:
                    emit_tail(*pend)
                while b_done:
                    emit_b_tail(*b_done.pop(0))

                # ---------- epilogue: reduce partials, emit num/den ----------
                nd = apool.tile([128, 4], dt.float32)  # cols (h, k): num,den
                nc.vector.tensor_reduce(
                    nd[:, 0:4:2],
                    num_parts[:].rearrange("q (h r) -> q h r", h=2),
                    mybir.AxisListType.X, ALU.add,
                )
                nc.vector.tensor_reduce(
                    nd[:, 1:4:2],
                    den_parts[:].rearrange("q (h r) -> q h r", h=2),
                    mybir.AxisListType.X, ALU.add,
                )

                nc.sync.dma_start(
                    out_t[:].rearrange("(h q) k -> q h k", h=2),
                    nd[:].rearrange("q (h k) -> q h k", h=2),
                )

    if split_waits:
        _split_multiwait(nc)
    return nc


def pack_weights(attn_w, attn_b, proj_w, fc_w):
    """Host-side packing of the tiny weights into device-ready layouts."""
    attn_w = np.asarray(attn_w, np.float32)
    attn_b = np.asarray(attn_b, np.float32).reshape(A)
    proj_w = np.asarray(proj_w, np.float32).reshape(A)
    fc_w = np.asarray(fc_w, np.float32).reshape(D)
    wp16 = np.zeros((128, WP_COLS), np.float16)
    wp16[0:D, 0:A] = attn_w.astype(np.float16)
    wp16[D:2 * D, 0:A] = attn_w.astype(np.float16)
    for m in range(4):
        wp16[32 * m:32 * (m + 1), A + m] = proj_w.astype(np.float16)
    # B-phase: wtil cols sorted non-negative-proj first, |proj| folded in
    # (sign carried by max-vs-min clamp); thr = -proj*attn_b per col.
    order = np.concatenate(
        [np.where(proj_w >= 0)[0], np.where(proj_w < 0)[0]])
    wtil = (attn_w * proj_w[None, :])[:, order].astype(np.float16)
    for hh in range(2):  # (k, hh)-interleaved cols, block-diagonal rows
        wp16[64 * hh:64 * (hh + 1), 36 + hh:36 + 64:2] = wtil
    for hh in range(2):
        wp16[64 * hh:64 * (hh + 1), 100 + hh] = fc_w.astype(np.float16)
    thr = (-(proj_w * attn_b))[order].astype(np.float16)
    wp16[:, 102:166:2] = np.tile(thr, (128, 1))
    wp16[:, 103:166:2] = np.tile(thr, (128, 1))
    wp32 = np.tile(attn_b, 4).reshape(128, 1).astype(np.float32)
    return wp16, wp32


def proj_na(proj_w):
    """Number of non-negative proj entries (B clamp split point)."""
    return int((np.asarray(proj_w, np.float32).reshape(-1) >= 0).sum())


def pack_x(x):
    """Host-side transpose + fp16 cast of x into the device layout:
    per core [128=(half,d), (field, b_q)]."""
    x = np.asarray(x, np.float32).reshape(N_CORES, N_HALF, BQ, F, D)
    xt = np.transpose(x.astype(np.float16), (0, 1, 4, 3, 2))
    return np.ascontiguousarray(xt.reshape(N_CORES * 128, F * BQ))


_CACHED = {}


def _get_runner(na=16):
    key = ("runner", na)
    if key in _CACHED:
        return _CACHED[key]
    import jax
    from jax.sharding import Mesh, PartitionSpec
    from jax.experimental.shard_map import shard_map
    import concourse.mybir as mybir
    from concourse.bass2jax import (
        _bass_exec_p, install_neuronx_cc_hook, partition_id_tensor,
    )

    nc = build_core_program(na=na)
    install_neuronx_cc_hook()

    partition_name = nc.partition_id_tensor.name if nc.partition_id_tensor else None
    in_names, out_names, out_avals, zero_outs = [], [], [], []
    for alloc in nc.m.functions[0].allocations:
        if not isinstance(alloc, mybir.MemoryLocationSet):
            continue
        name = alloc.memorylocations[0].name
        if alloc.kind == "ExternalInput":
            if name != partition_name:
                in_names.append(name)
        elif alloc.kind == "ExternalOutput":
            out_names.append(name)
            shape = tuple(alloc.tensor_shape)
            dtype = mybir.dt.np(alloc.dtype)
            out_avals.append(jax.core.ShapedArray(shape, dtype))
            zero_outs.append(np.zeros(shape, dtype))
    n_params = len(in_names)
    n_outs = len(out_avals)
    all_in = in_names + out_names + ([partition_name] if partition_name else [])

    def _body(*args):
        operands = list(args)
        if partition_name is not None:
            operands.append(partition_id_tensor())
        outs = _bass_exec_p.bind(
            *operands,
            out_avals=tuple(out_avals),
            in_names=tuple(all_in),
            out_names=tuple(out_names),
            lowering_input_output_aliases=(),
            sim_require_finite=True,
            sim_require_nnan=True,
            nc=nc,
        )
        return tuple(outs)

    devices = jax.devices()[:N_CORES]
    mesh = Mesh(np.asarray(devices), ("core",))
    fn = jax.jit(
        shard_map(
            _body, mesh=mesh,
            in_specs=(PartitionSpec("core"),) * (n_params + n_outs),
            out_specs=(PartitionSpec("core"),) * n_outs,
            check_rep=False,
        ),
        keep_unused=True,
    )
    _CACHED[key] = {
        "fn": fn, "in_names": in_names, "out_names": out_names,
        "zero_outs": zero_outs, "mesh": mesh, "nc": nc,
    }
    return _CACHED[key]


def _device_args(r, x, attn_w, attn_b, proj_w, fc_w):
    wp16, wp32 = pack_weights(attn_w, attn_b, proj_w, fc_w)
    feeds = {
        "x": pack_x(x),
        "wp16": np.ascontiguousarray(np.tile(wp16, (N_CORES, 1))),
        "wp32": np.ascontiguousarray(np.tile(wp32, (N_CORES, 1))),
    }
    concat_in = [feeds[n] for n in r["in_names"]]
    concat_zeros = [
        np.zeros((N_CORES * z.shape[0], *z.shape[1:]), z.dtype)
        for z in r["zero_outs"]
    ]
    return concat_in + concat_zeros


def kernel(x, attn_w, attn_b, proj_w, proj_b, fc_w, fc_b):
    """FULL inputs -> FULL output. proj_b is softmax-invariant (unused)."""
    import jax

    r = _get_runner(na=proj_na(proj_w))
    args = _device_args(r, x, attn_w, attn_b, proj_w, fc_w)
    outs = r["fn"](*args)
    jax.block_until_ready(outs)
    nd = np.asarray(outs[r["out_names"].index("out")]).reshape(B, 2)
    fc_b = np.asarray(fc_b, np.float32)
    return (nd[:, 0] / nd[:, 1] + fc_b[0]).astype(np.float32)[:, None]



# revision 27
# speedup vs baseline: 1.3814x; 1.3814x over previous
"""Trainium2 Bass kernel for AttentionalFactorizationMachine.

kernel(**inputs) takes FULL unsharded inputs, returns FULL [2048, 1] output.
Internally: data-parallel over 8 NeuronCores (batch sharded, weights
replicated), one SPMD Bass program.

Per-core algorithm (256 items, 780 pairs padded to 784):
  out[b] = (sum_p E_p * g_p) / (sum_p E_p) + fc_b
    E_p = exp(l_p)                       [proj_b dropped: softmax-invariant]
    l_p = proj_w . relu(attn_w^T (x_i*x_j) + attn_b)
    g_p = fc_w . (x_i*x_j)
  Device computes num/den per item; host does the final divide + fc_b.

Layouts (SBUF [partition, free]):
  X_T [ (half,d)=128, (field,b_q)=40*128 ] fp16  HOST-packed (one DMA)
  ip  [ (half,d)=128, (pair_loc,b_q)     ] fp16  DVE broadcast tensor_mul (2x)
Pipeline per supertile (16 pairs):
  L1:  8 concurrent matmuls (2 b-halves x 4 col-groups, tile_position),
       lhsT=attn_w -> ps_t2 [(cgrp,a)=128, 1024]
  ACT: fused bias+relu psum->SBUF hs2 fp16
  L2': hs2 128-col chunks as lhsT (transposed trick), rhs=proj4 [128,4]
       -> lg_ps [b_q=128, pair-cols]   (dense logits)  -- LDW ~free (FWL)
  g:   ip 128-col chunks as lhsT, rhs=fcw2 [128,2]
       -> g_ps [b_q=128, pair-cols]    (dense values)
Software pipeline: the L2'/g matmuls of supertile s are EMITTED after the
L1 of supertile s+1.  They read hs2 (the ACT relu output); without the
delay the in-order PE queue stalls ~1us on the ACT every supertile --
that stall, not matmul cost, dominated earlier versions.
Per round (16 supertiles): ACT exp -> E + accum_out (den partials);
DVE scalar_tensor_tensor(E*g) -> num partials (chained).
"""

import numpy as np

B, F, D, A = 2048, 40, 64, 32
N_CORES = 8
BC = B // N_CORES          # 256 items per core
BQ = 128                   # items per half
N_HALF = 2
PAIRS = F * (F - 1) // 2   # 780
ST_PAIRS = 16
N_ST = (PAIRS + ST_PAIRS - 1) // ST_PAIRS       # 49
PAIRS_PAD = N_ST * ST_PAIRS                     # 784
ROUND_ST = 16
N_ROUNDS = (N_ST + ROUND_ST - 1) // ROUND_ST    # 4
NEG_BIG = -1.0e30
IP_BUFS = 4
H_BUFS = 4
WP_COLS = 166      # [aw 32 | proj4 4 | wB 64+2(shared fc) | thr 64]
A_ST = 49          # supertiles on the A pipeline; pairs >= 16*A_ST go via B
                   # (A_ST=49: pure-A. The B pipeline measured slower on HW:
                   # per-pair LDW+66col MM + DVE clamp chains didn't pipeline
                   # as modeled; kept for reference with A_ST<49.)
B_CH = 14          # pairs per B chunk (7 per PSUM bank, 66 fp32 cols each)

# pairs ordered by (j, i): long i-runs per j => few DVE ops
_ROWJ = np.array([i for j in range(1, F) for i in range(j)], np.int32)
_COLJ = np.array([j for j in range(1, F) for i in range(j)], np.int32)


def _pos(p_local: int) -> int:
    # L2' chunk j with rhs col m' produces pair_local=4m'+j at col 4j+m'
    return 4 * (p_local % 4) + p_local // 4


def _b_chunks(a_st):
    """B-phase chunks [(start_pair, npairs), ...] covering [16*a_st, 780)."""
    out, p = [], ST_PAIRS * a_st
    while p < PAIRS:
        n = min(B_CH, PAIRS - p)
        out.append((p, n))
        p += n
    return out


def _patch_tile_drain():
    """This walrus build accepts only ONE sync wait per instruction; split the
    TileContext exit drain into a chain of single-wait drains."""
    import bass_rust
    import concourse.tile as tile_mod
    from concourse.tile import TileContext

    if getattr(TileContext, "_drain_patched", False):
        return

    def _drain_and_barrier(self, tick_clock, wait_clock):
        drain_inst = self.nc.sync.drain()
        wait_clock.add_sem_waits(
            drain_inst.ins, tile_mod.ScopedClock({None: tick_clock.global_clock})
        )
        si = drain_inst.ins.sync_info
        if si is not None and len(si.on_wait) > 1:
            waits = list(si.on_wait)
            drain_inst.ins.sync_info = bass_rust.SyncInfo(
                on_wait=[waits[0]], on_update=list(si.on_update)
            )
            for w in waits[1:]:
                extra = self.nc.sync.drain()
                extra.ins.sync_info = bass_rust.SyncInfo(on_wait=[w], on_update=[])

    TileContext._drain_and_barrier = _drain_and_barrier
    TileContext._drain_patched = True


def _split_multiwait(nc):
    """Walrus here allows ONE sync wait per instruction: move surplus waits
    onto same-engine NoOps inserted immediately before the instruction."""
    import concourse.mybir as mybir

    for f in nc.m.functions:
        for blk in f.blocks:
            il = blk.instructions
            idx = 0
            while idx < len(il):
                inst = il[idx]
                si = inst.sync_info
                if si is not None and len(si.on_wait) > 1:
                    waits = list(si.on_wait)
                    inst.sync_info = mybir.SyncInfo(
                        on_wait=[waits[-1]], on_update=list(si.on_update)
                    )
                    for k, w in enumerate(waits[:-1]):
                        nop = mybir.InstNoOp(
                            name=f"{inst.name}_w{k}",
                            sync_info=mybir.SyncInfo(on_wait=[w], on_update=[]),
                            bass_nofuse=True,
                            engine=inst.engine,
                        )
                        il.insert(idx, nop)
                        idx += 1
                idx += 1


def _seg_of_st():
    """Per-supertile ip-build segments: ("tt", j, i0, i1, p_local0) or
    ("pad", 0, 0, n, pl); i-runs split only at supertile boundaries."""
    seg_of_st = []
    for s in range(N_ST):
        p_lo, p_hi = s * ST_PAIRS, (s + 1) * ST_PAIRS
        segs, p = [], p_lo
        while p < p_hi:
            if p < PAIRS:
                i, j = int(_ROWJ[p]), int(_COLJ[p])
                run = min(p_hi, PAIRS, p + (j - i)) - p  # i..j-1 within j-row
                segs.append(("tt", j, i, i + run, p - p_lo))
                p += run
            else:
                segs.append(("pad", 0, 0, p_hi - p, p - p_lo))
                p = p_hi
        seg_of_st.append(segs)
    return seg_of_st


def build_core_program(split_waits=True, repeat=1, skip=(), a_st=A_ST, na=16):
    """The single-core SPMD Bass program (identical on all 8 cores).

    a_st: supertiles handled by the A pipeline (relu on ACT, L2'/g LDW
    matmuls); pairs >= 16*a_st go through the B pipeline (per-pair
    LDW+66col matmul, sign-folded clamp on DVE).  na: number of
    non-negative proj_w entries (clamp split point; affects values only,
    not timing).
    """
    import concourse.bass as bass
    import concourse.mybir as mybir
    from concourse.tile import TileContext

    _patch_tile_drain()
    dt = mybir.dt
    AF = mybir.ActivationFunctionType
    ALU = mybir.AluOpType

    nc = bass.Bass()
    # x arrives host-transposed + fp16: [(half,d)=128, (field, b_q)]
    x_in = nc.dram_tensor("x", [128, F * BQ], dt.float16,
                          kind="ExternalInput")
    # wp16 [128, 134] fp16 = [aw(32) | proj4(4) | wB(64) | fcw2(2) | thr(32)]
    # wp32 [128, 1] fp32 = attn_b tiled 4x
    wp16_in = nc.dram_tensor("wp16", [128, WP_COLS], dt.float16,
                             kind="ExternalInput")
    wp32_in = nc.dram_tensor("wp32", [128, 1], dt.float32, kind="ExternalInput")
    out_t = nc.dram_tensor("out", [BC, 2], dt.float32, kind="ExternalOutput")

    seg_of_st = _seg_of_st()
    n_rounds_a = (a_st + ROUND_ST - 1) // ROUND_ST
    bchunks = _b_chunks(a_st)
    nb_ch = len(bchunks)
    # ragged 2-bank chunk needs a second num partial column
    extra_col = 1 if any(7 < n < B_CH for _, n in bchunks) else 0
    npart = n_rounds_a + nb_ch + extra_col

    with TileContext(nc) as tc:
        with (
            tc.tile_pool(name="const", bufs=1) as cpool,
            tc.tile_pool(name="xstage", bufs=1) as xpool,
            tc.tile_pool(name="ip", bufs=IP_BUFS) as ippool,
            tc.tile_pool(name="hbuf", bufs=H_BUFS) as hpool,
            tc.tile_pool(name="escr", bufs=2) as epool,
            tc.tile_pool(name="bbuf", bufs=3) as bpool,
            tc.tile_pool(name="acc", bufs=1) as apool,
            tc.tile_pool(name="pst", bufs=2, space="PSUM") as pst,
            tc.tile_pool(name="plg", bufs=2, space="PSUM") as plg,
            tc.tile_pool(name="pgv", bufs=2, space="PSUM") as pgv,
        ):
            import contextlib
            loop_cm = (tc.For_i(0, repeat, 1) if repeat > 1
                       else contextlib.nullcontext())
            with loop_cm:
                # ---------- weights + x: 3 DMAs ----------
                wp16 = cpool.tile([128, WP_COLS], dt.float16)
                nc.sync.dma_start(wp16[:], wp16_in[:])
                ab = cpool.tile([128, 1], dt.float32)
                nc.sync.dma_start(ab[:], wp32_in[:])
                aw = wp16[:, 0:A]
                proj4 = wp16[:, A:A + 4]
                wB = wp16[:, 36:102]        # [wtil(64, (k,hh)) | fc(2)]
                fcw2 = wp16[:, 100:102]
                thr_t = wp16[:, 102:166]    # clamp thresholds, all-partition
                xt = xpool.tile([128, F * BQ], dt.float16, tag="xt")
                nc.sync.dma_start(xt[:], x_in[:])

                # ---------- accumulators ----------
                # partials laid out [128, (h, r)]: col h*npart + r
                num_parts = apool.tile([128, 2 * npart], dt.float32)
                den_parts = apool.tile([128, 2 * npart], dt.float32)
                nc.vector.memset(num_parts[:], 0.0)
                nc.vector.memset(den_parts[:], 0.0)

                # round r -> (lg_ps, g_ps); delayed by the sw pipeline
                round_ps = {}
                pend = None  # (s, r, sl, ip, hs2)

                def emit_tail(s0, r0, sl0, ip0, hs20):
                    lg_ps, g_ps = round_ps[r0]
                    # ---- L2' dense logits (lhsT = hs2 data, LDW ~free) ----
                    for jj in (range(8)
                               if not ({"lg", "l2"} & set(skip)) else []):
                        h, j = jj // 4, jj % 4
                        c0 = 32 * sl0 + 16 * h + 4 * j
                        nc.tensor.matmul(
                            lg_ps[:, c0:c0 + 4],
                            hs20[:, 512 * h + 128 * j:512 * h + 128 * (j + 1)],
                            proj4,
                            start=True, stop=True,
                        )
                    # ---- g dense ----
                    for pl in (range(ST_PAIRS)
                               if not ({"lg", "g"} & set(skip)) else []):
                        c0 = 32 * sl0 + 2 * _pos(pl)
                        nc.tensor.matmul(
                            g_ps[:, c0:c0 + 2],
                            ip0[:, BQ * pl:BQ * (pl + 1)],
                            fcw2,
                            start=True, stop=True,
                        )
                    st1 = min((r0 + 1) * ROUND_ST, a_st)
                    if s0 != st1 - 1 or "red" in skip:
                        return
                    n_st_r = st1 - r0 * ROUND_ST
                    # ---- poison pad logits -> exp gives 0 ----
                    if st1 == N_ST and PAIRS_PAD > PAIRS:
                        sl = (N_ST - 1) - r0 * ROUND_ST
                        for h in range(N_HALF):
                            base = 32 * sl + 16 * h
                            nc.vector.memset(
                                lg_ps[:, base + 3:base + 16:4], NEG_BIG
                            )
                    # ---- exp (den) + E*g reduce (num) ----
                    e_sb = epool.tile([128, 512], dt.float32, tag="E",
                                      name=f"e_{r0}")
                    tt_scr = epool.tile([128, 256], dt.float32, tag="ttscr",
                                        name=f"tt_{r0}")
                    for h in range(N_HALF):
                        l_ap = lg_ps[:].rearrange(
                            "q (s hh c) -> q c hh s", hh=2, c=16
                        )[:, :, h, 0:n_st_r]
                        e_ap = e_sb[:].rearrange(
                            "q (s hh c) -> q c hh s", hh=2, c=16
                        )[:, :, h, 0:n_st_r]
                        g_ap = g_ps[:].rearrange(
                            "q (s c hh) -> q c hh s", c=16, hh=2
                        )[:, :, h, 0:n_st_r]
                        col = h * npart + r0
                        nc.scalar.activation(
                            e_ap, l_ap, AF.Exp,
                            accum_out=den_parts[:, col:col + 1],
                        )
                        nc.vector.scalar_tensor_tensor(
                            out=tt_scr[:, 0:16 * n_st_r].rearrange(
                                "q (c s) -> q c s", s=n_st_r
                            ),
                            in0=e_ap,
                            scalar=1.0,
                            in1=g_ap,
                            op0=ALU.mult,
                            op1=ALU.mult,
                            accum_out=num_parts[:, col:col + 1],
                        )

                def emit_b_tail(bp, p0, n, ci):
                    """Clamp+reduce+exp+num for one B chunk of n pairs."""
                    hb = hpool.tile([128, 1024], dt.float16, tag="h",
                                    name=f"hb_{ci}")
                    banks = [(0, min(n, 7)), (1, max(n - 7, 0))]
                    for b, nbk in banks:
                        if nbk:
                            nc.scalar.activation(
                                hb[:, 512 * b:512 * b + 66 * nbk],
                                bp[:, 512 * b:512 * b + 66 * nbk],
                                AF.Copy)
                    # sign-folded clamp: col k<na -> max(h~, thr), else min
                    # wB/hb/hc h-cols are (k, hh)-interleaved: col 2k+hh
                    hc = bpool.tile([128, 64 * B_CH], dt.float16, tag="hc",
                                    name=f"hc_{ci}")
                    for b, nbk in banks:
                        if not nbk:
                            continue
                        h_in = hb[:, 512 * b:512 * b + 66 * nbk].rearrange(
                            "q (pp c) -> q pp c", c=66)
                        h_out = hc[:, 448 * b:448 * b + 64 * nbk].rearrange(
                            "q (pp c) -> q pp c", c=64)
                        for k0, k1, op in ((0, na, ALU.max),
                                           (na, 32, ALU.min)):
                            if k1 <= k0:
                                continue
                            nc.vector.scalar_tensor_tensor(
                                out=h_out[:, :, 2 * k0:2 * k1],
                                in0=h_in[:, :, 2 * k0:2 * k1],
                                scalar=1.0,
                                in1=thr_t[:, 2 * k0:2 * k1].rearrange(
                                    "q (o c) -> q o c", o=1
                                ).broadcast_to((128, nbk, 2 * (k1 - k0))),
                                op0=ALU.mult,
                                op1=op,
                            )
                    # l[q, (slot, hh)] = sum over the 32 clamped cols
                    rd = bpool.tile([128, 2 * B_CH], dt.float32, tag="rd",
                                    name=f"rd_{ci}")
                    for h in range(N_HALF):
                        nc.vector.tensor_reduce(
                            rd[:, 0:2 * n].rearrange(
                                "q (sl hh) -> q sl hh", hh=2)[:, :, h],
                            hc[:, 0:64 * n].rearrange(
                                "q (sl kh) -> q sl kh", kh=64)[:, :, h::2],
                            mybir.AxisListType.X, ALU.add,
                        )
                    if "red" in skip:
                        return
                    e_b = bpool.tile([128, 2 * B_CH], dt.float32, tag="eb",
                                     name=f"eb_{ci}")
                    tt_b = bpool.tile([128, B_CH], dt.float32, tag="ttb",
                                      name=f"ttb_{ci}")
                    for h in range(N_HALF):
                        col = h * npart + n_rounds_a + ci
                        e_ap = e_b[:, 0:2 * n].rearrange(
                            "q (sl hh) -> q sl hh", hh=2)[:, :, h]
                        nc.scalar.activation(
                            e_ap,
                            rd[:, 0:2 * n].rearrange(
                                "q (sl hh) -> q sl hh", hh=2)[:, :, h],
                            AF.Exp,
                            accum_out=den_parts[:, col:col + 1],
                        )
                        if n == B_CH:
                            nc.vector.scalar_tensor_tensor(
                                out=tt_b[:, 0:B_CH].rearrange(
                                    "q (b pp) -> q b pp", b=2),
                                in0=e_b[:, 0:2 * n].rearrange(
                                    "q (b pp hh) -> q b pp hh", b=2, hh=2
                                )[:, :, :, h],
                                scalar=1.0,
                                in1=hb[:].rearrange(
                                    "q (b r) -> q b r", b=2
                                )[:, :, 64 + h:64 + h + 66 * 6 + 1:66],
                                op0=ALU.mult, op1=ALU.mult,
                                accum_out=num_parts[:, col:col + 1],
                            )
                        else:
                            for b, nbk in banks:
                                if not nbk:
                                    continue
                                colb = col if b == 0 else h * npart + npart - 1
                                nc.vector.scalar_tensor_tensor(
                                    out=tt_b[:, 0:nbk],
                                    in0=e_b[:, 0:2 * n].rearrange(
                                        "q (sl hh) -> q sl hh", hh=2
                                    )[:, 7 * b:7 * b + nbk, h],
                                    scalar=1.0,
                                    in1=hb[:, 512 * b + 64 + h:
                                           512 * b + 64 + h
                                           + 66 * (nbk - 1) + 1:66],
                                    op0=ALU.mult, op1=ALU.mult,
                                    accum_out=num_parts[:, colb:colb + 1],
                                )

                b_done = []   # chunks awaiting their tail (delay-by-one)
                b_cur = None  # [bp_tile, start_pair, chunk_idx]

                for s in range(N_ST):
                    is_a = s < a_st
                    r, sl = divmod(s, ROUND_ST)
                    if is_a and sl == 0:
                        lg_ps = plg.tile([128, 512], dt.float32, tag="lg",
                                         name=f"lg_{r}")
                        g_ps = pgv.tile([128, 512], dt.float32, tag="gv",
                                        name=f"g_{r}")
                        round_ps[r] = (lg_ps, g_ps)

                    # ---- ip build ----
                    ip = ippool.tile([128, ST_PAIRS * BQ], dt.float16,
                                     tag="ip", name=f"ip_{s}")
                    segs = seg_of_st[s] if "ip" not in skip else []
                    if not is_a:
                        segs = [sg for sg in segs if sg[0] == "tt"]
                    if "ip" in skip:
                        nc.vector.tensor_mul(
                            ip[:, 0:BQ], xt[:, 0:BQ], xt[:, BQ:2 * BQ])
                    for kind, j, i0, i1, pl in segs:
                        if kind == "pad":
                            nc.vector.memset(
                                ip[:, pl * BQ:(pl + i1 - i0) * BQ], 0.0)
                            continue
                        nii = i1 - i0
                        nc.vector.tensor_mul(
                            ip[:, pl * BQ:(pl + nii) * BQ].rearrange(
                                "p (j q) -> p j q", j=nii
                            ),
                            xt[:, i0 * BQ:i1 * BQ].rearrange(
                                "p (j q) -> p j q", j=nii
                            ),
                            xt[:, j * BQ:(j + 1) * BQ].rearrange(
                                "p (o q) -> p o q", o=1
                            ).broadcast_to((128, nii, BQ)),
                        )

                    if is_a:
                        # ---- L1: 8 concurrent matmuls, one 2-bank tile ----
                        ps_t2 = pst.tile([128, 1024], dt.float32, tag="t",
                                         name=f"ps_t_{s}")
                        for h in (range(N_HALF) if "l1" not in skip else []):
                            for c in range(4):
                                nc.tensor.matmul(
                                    ps_t2[32 * c:32 * (c + 1),
                                          512 * h:512 * (h + 1)],
                                    aw[64 * h:64 * (h + 1), :],
                                    ip[64 * h:64 * (h + 1),
                                       512 * c:512 * (c + 1)],
                                    start=True, stop=True,
                                    tile_position=(64 * h, 32 * c),
                                )

                        # ---- relu + bias ----
                        hs2 = hpool.tile([128, 1024], dt.float16, tag="h",
                                         name=f"hs_{s}")
                        if "relu" in skip:
                            nc.scalar.activation(
                                hs2[:, 0:16], ps_t2[:, 0:16], AF.Relu,
                                bias=ab[:], scale=1.0)
                        else:
                            nc.scalar.activation(
                                hs2[:], ps_t2[:], AF.Relu, bias=ab[:],
                                scale=1.0)

                        # ---- delayed L2'/g of the PREVIOUS supertile ----
                        if pend is not None:
                            emit_tail(*pend)
                        pend = (s, r, sl, ip, hs2)
                    elif "l1" not in skip:
                        # ---- B phase: per-pair LDW+66col matmul ----
                        if pend is not None:
                            emit_tail(*pend)
                            pend = None
                        for pl in range(ST_PAIRS):
                            pair = ST_PAIRS * s + pl
                            if pair >= PAIRS:
                                break
                            bi = pair - ST_PAIRS * a_st
                            ci, slot = divmod(bi, B_CH)
                            if slot == 0:
                                bp = pst.tile([128, 1024], dt.float32,
                                              tag="t", name=f"bp_{ci}")
                                b_cur = [bp, pair, ci]
                            c0 = 512 * (slot // 7) + 66 * (slot % 7)
                            nc.tensor.matmul(
                                b_cur[0][:, c0:c0 + 66],
                                ip[:, BQ * pl:BQ * (pl + 1)],
                                wB,
                                start=True, stop=True,
                            )
                            if slot == B_CH - 1 or pair == PAIRS - 1:
                                # tail of the PREVIOUS chunk (delay-by-one)
                                while b_done:
                                    emit_b_tail(*b_done.pop(0))
                                b_done.append(
                                    (b_cur[0], b_cur[1],
                                     pair - b_cur[1] + 1, b_cur[2]))

                # flush the last supertile / B chunks
                if pend is not None:
                    emit_tail(*pend)
                while b_done:
                    emit_b_tail(*b_done.pop(0))

                # ---------- epilogue: reduce partials, emit num/den ----------
                nd = apool.tile([128, 4], dt.float32)  # cols (h, k): num,den
                nc.vector.tensor_reduce(
                    nd[:, 0:4:2],
                    num_parts[:].rearrange("q (h r) -> q h r", h=2),
                    mybir.AxisListType.X, ALU.add,
                )
                nc.vector.tensor_reduce(
                    nd[:, 1:4:2],
                    den_parts[:].rearrange("q (h r) -> q h r", h=2),
                    mybir.AxisListType.X, ALU.add,
                )

                nc.sync.dma_start(
                    out_t[:].rearrange("(h q) k -> q h k", h=2),
                    nd[:].rearrange("q (h k) -> q h k", h=2),
                )

    if split_waits:
        _split_multiwait(nc)
    return nc


def pack_weights(attn_w, attn_b, proj_w, fc_w):
    """Host-side packing of the tiny weights into device-ready layouts."""
    attn_w = np.asarray(attn_w, np.float32)
    attn_b = np.asarray(attn_b, np.float32).reshape(A)
    proj_w = np.asarray(proj_w, np.float32).reshape(A)
    fc_w = np.asarray(fc_w, np.float32).reshape(D)
    wp16 = np.zeros((128, WP_COLS), np.float16)
    wp16[0:D, 0:A] = attn_w.astype(np.float16)
    wp16[D:2 * D, 0:A] = attn_w.astype(np.float16)
    for m in range(4):
        wp16[32 * m:32 * (m + 1), A + m] = proj_w.astype(np.float16)
    # B-phase: wtil cols sorted non-negative-proj first, |proj| folded in
    # (sign carried by max-vs-min clamp); thr = -proj*attn_b per col.
    order = np.concatenate(
        [np.where(proj_w >= 0)[0], np.where(proj_w < 0)[0]])
    wtil = (attn_w * proj_w[None, :])[:, order].astype(np.float16)
    for hh in range(2):  # (k, hh)-interleaved cols, block-diagonal rows
        wp16[64 * hh:64 * (hh + 1), 36 + hh:36 + 64:2] = wtil
    for hh in range(2):
        wp16[64 * hh:64 * (hh + 1), 100 + hh] = fc_w.astype(np.float16)
    thr = (-(proj_w * attn_b))[order].astype(np.float16)
    wp16[:, 102:166:2] = np.tile(thr, (128, 1))
    wp16[:, 103:166:2] = np.tile(thr, (128, 1))
    wp32 = np.tile(attn_b, 4).reshape(128, 1).astype(np.float32)
    return wp16, wp32


def proj_na(proj_w):
    """Number of non-negative proj entries (B clamp split point)."""
    return int((np.asarray(proj_w, np.float32).reshape(-1) >= 0).sum())


def pack_x(x):
    """Host-side transpose + fp16 cast of x into the device layout:
    per core [128=(half,d), (field, b_q)]."""
    x = np.asarray(x, np.float32).reshape(N_CORES, N_HALF, BQ, F, D)
    xt = np.transpose(x.astype(np.float16), (0, 1, 4, 3, 2))
    return np.ascontiguousarray(xt.reshape(N_CORES * 128, F * BQ))


_CACHED = {}


def _get_runner(na=16):
    key = ("runner", na)
    if key in _CACHED:
        return _CACHED[key]
    import jax
    from jax.sharding import Mesh, PartitionSpec
    from jax.experimental.shard_map import shard_map
    import concourse.mybir as mybir
    from concourse.bass2jax import (
        _bass_exec_p, install_neuronx_cc_hook, partition_id_tensor,
    )

    nc = build_core_program(na=na)
    install_neuronx_cc_hook()

    partition_name = nc.partition_id_tensor.name if nc.partition_id_tensor else None
    in_names, out_names, out_avals, zero_outs = [], [], [], []
    for alloc in nc.m.functions[0].allocations:
        if not isinstance(alloc, mybir.MemoryLocationSet):
            continue
        name = alloc.memorylocations[0].name
        if alloc.kind == "ExternalInput":
            if name != partition_name:
                in_names.append(name)
        elif alloc.kind == "ExternalOutput":
            out_names.append(name)
            shape = tuple(alloc.tensor_shape)
            dtype = mybir.dt.np(alloc.dtype)
            out_avals.append(jax.core.ShapedArray(shape, dtype))
            zero_outs.append(np.zeros(shape, dtype))
    n_params = len(in_names)
    n_outs = len(out_avals)
    all_in = in_names + out_names + ([partition_name] if partition_name else [])

    def _body(*args):
        operands = list(args)
        if partition_name is not None:
            operands.append(partition_id_tensor())
        outs = _bass_exec_p.bind(
            *operands,
            out_avals=tuple(out_avals),
            in_names=tuple(all_in),
            out_names=tuple(out_names),
            lowering_input_output_aliases=(),
            sim_require_finite=True,
            sim_require_nnan=True,
            nc=nc,
        )
        return tuple(outs)

    devices = jax.devices()[:N_CORES]
    mesh = Mesh(np.asarray(devices), ("core",))
    fn = jax.jit(
        shard_map(
            _body, mesh=mesh,
            in_specs=(PartitionSpec("core"),) * (n_params + n_outs),
            out_specs=(PartitionSpec("core"),) * n_outs,
            check_rep=False,
        ),
        keep_unused=True,
    )
    _CACHED[key] = {
        "fn": fn, "in_names": in_names, "out_names": out_names,
        "zero_outs": zero_outs, "mesh": mesh, "nc": nc,
    }
    return _CACHED[key]


def _device_args(r, x, attn_w, attn_b, proj_w, fc_w):
    wp16, wp32 = pack_weights(attn_w, attn_b, proj_w, fc_w)
    feeds = {
        "x": pack_x(x),
        "wp16": np.ascontiguousarray(np.tile(wp16, (N_CORES, 1))),
        "wp32": np.ascontiguousarray(np.tile(wp32, (N_CORES, 1))),
    }
    concat_in = [feeds[n] for n in r["in_names"]]
    concat_zeros = [
        np.zeros((N_CORES * z.shape[0], *z.shape[1:]), z.dtype)
        for z in r["zero_outs"]
    ]
    return concat_in + concat_zeros


def kernel(x, attn_w, attn_b, proj_w, proj_b, fc_w, fc_b):
    """FULL inputs -> FULL output. proj_b is softmax-invariant (unused)."""
    import jax

    r = _get_runner(na=proj_na(proj_w))
    args = _device_args(r, x, attn_w, attn_b, proj_w, fc_w)
    outs = r["fn"](*args)
    jax.block_until_ready(outs)
    nd = np.asarray(outs[r["out_names"].index("out")]).reshape(B, 2)
    fc_b = np.asarray(fc_b, np.float32)
    return (nd[:, 0] / nd[:, 1] + fc_b[0]).astype(np.float32)[:, None]

